# revision 32
# baseline (speedup 1.0000x reference)
"""Trainium2 Bass kernel for nn_LocalEncoder (masked GRU + attention pooling).

Strategy (v3):
- Data-parallel over batch: 8 cores x 512 rows. Rows are length-sorted and
  dealt round-robin so every core gets an identical length profile, then
  split into 4 chunks of 128 (short->long). Chunk c only scans T_c steps
  (T_c = max length in chunk, uniform across cores) - ~25% less work.
- Feature-major [U partitions, batch free]. All matmuls bf16 -> fp32 PSUM.
- Scan: per chunk-step one PSUM bank holds [z|r|xh|rh]. Emission order is
  latency-first: rec matmuls for step t (longest chunk first) BEFORE the
  x-side matmuls for t+1, so the serial recurrence chain is never queued
  behind slack work. The per-step chain is
    rec -> sigmoid -> t1 -> iacc -> tanh -> u,stw -> rec(t+1)
  with the z-blend split as  h_t = u - negw,  u = z*hh (DVE, on-chain),
  negw = (z-1)*h_{t-1} (GpSimd, computed in the tanh shadow, off-chain).
  Trailing-padding mask folded via -40 * (1-m) row into the z-gate.
- All state stays in SBUF: stage[c] = [100, T_c, BC] bf16.
- Attention: sigmoid(A1*last + A2*state_t) with the A1 term applied for
  ALL t; host subtracts the closed-form correction for masked steps and
  adds the contribution of steps beyond T_c. 4-step groups, c1 added via
  a PE identity-accumulate (not DVE), alpha written back over the same
  PSUM bank by the Vr matmul, alpha*state on DVE, accumulation on GpSimd.
  Groups are pumped into engine slack during the scan (budget scales with
  the number of retired chunks) and drained 4-pools-deep afterwards.
"""
import sys
sys.path.insert(0, "/opt/trn_rl_repo")
from contextlib import ExitStack

import numpy as np
import ml_dtypes

import concourse.bass as bass
import concourse.bacc as bacc
import concourse.tile as tile
from concourse import mybir
from concourse import bass_utils

bf16 = ml_dtypes.bfloat16
AF = mybir.ActivationFunctionType
OP = mybir.AluOpType

B, T, E, U = 4096, 200, 100, 100
NCORES = 8
NCHUNK = 4
BC = 128
PERCORE = NCHUNK * BC
GL = 4  # attention group length (steps per group); 1 PSUM bank per group

_CACHE = {}


def _ceil8(x):
    return min(((int(x) + 7) // 8) * 8, T)


def _build(Ts):
    """Ts: per-chunk step counts (uniform across cores)."""
    nc = bacc.Bacc()
    dt = mybir.dt

    xcs = [nc.dram_tensor(f"xc{c}", [128, Ts[c], BC], dt.bfloat16,
                          kind="ExternalInput") for c in range(NCHUNK)]
    # all stationary weights pre-padded to [128,128] host-side, one DMA
    wall_d = nc.dram_tensor("wall", [128, 12, 128], dt.bfloat16,
                            kind="ExternalInput")
    ones_d = nc.dram_tensor("wones", [1, T, BC], dt.bfloat16,
                            kind="ExternalInput")
    lastout = [nc.dram_tensor(f"lastc{c}", [U, BC], dt.float32,
                              kind="ExternalOutput") for c in range(NCHUNK)]
    outp = [nc.dram_tensor(f"outp{c}", [U, BC], dt.float32,
                           kind="ExternalOutput") for c in range(NCHUNK)]

    maxT = max(Ts)
    ORDER = list(range(NCHUNK - 1, -1, -1))  # longest chunk first

    with tile.TileContext(nc) as tc, ExitStack() as octx:
        singles = octx.enter_context(tc.tile_pool(name="singles", bufs=1))
        xpool = octx.enter_context(tc.tile_pool(name="xpool", bufs=2))
        gp = octx.enter_context(tc.tile_pool(name="gp", bufs=3))
        bankp = [octx.enter_context(
            tc.tile_pool(name=f"bankp{c}", bufs=2, space="PSUM"))
            for c in range(NCHUNK)]

        wall = singles.tile([128, 12, 128], dt.bfloat16, tag="wall", name="wall")
        nc.sync.dma_start(out=wall, in_=wall_d[:, :, :])
        Kz, Kr, Kh = wall[:, 0, :], wall[:, 1, :], wall[:, 2, :]
        Rz, Rr, Rh = wall[0:101, 3, :], wall[0:101, 4, :], wall[0:101, 5, :]
        A1, A2 = wall[0:U, 6, :], wall[0:U, 7, :]
        Vr, I100 = wall[0:U, 8, :], wall[0:U, 9, :]
        b1h = wall[0:1, 10, :]
        b1hT = wall[0:U, 11, 0:1]
        ones = singles.tile([1, BC], dt.bfloat16, tag="ones")
        nc.vector.memset(ones, 1.0)

        stages = []
        for c in range(NCHUNK):
            # row 100 is a constant-ones row: the rec rhs is [101,BC] so the
            # Rh bias row folds b1h into the rh matmul (t1 becomes a plain TT)
            st = singles.tile([101, Ts[c], BC], dt.bfloat16, tag=f"stage{c}",
                              name=f"stage{c}")
            nc.sync.dma_start(out=st[100:101, :, :], in_=ones_d[:, 0:Ts[c], :])
            stages.append(st)

        xblks = [dict() for _ in range(NCHUNK)]
        banks = [dict() for _ in range(NCHUNK)]
        zrs_t = [None] * NCHUNK
        t1_t = [None] * NCHUNK
        negw_t = [None] * NCHUNK

        def issue_xdma(c, k):
            if k * 8 >= Ts[c]:
                return
            xt = xpool.tile([128, 8, BC], dt.bfloat16, tag=f"x{c}", name=f"xb{c}")
            nc.sync.dma_start(out=xt, in_=xcs[c][:, k * 8:(k + 1) * 8, :])
            xblks[c][k] = xt

        def recgroup(t):
            for c in ORDER:
                if t < 1 or t >= Ts[c]:
                    continue
                h = stages[c][0:101, t - 1, :]
                bk = banks[c][t]
                nc.tensor.matmul(bk[:, 0, :], lhsT=Rz, rhs=h, start=False, stop=True)
                nc.tensor.matmul(bk[:, 1, :], lhsT=Rr, rhs=h, start=False, stop=True)
                nc.tensor.matmul(bk[:, 3, :], lhsT=Rh, rhs=h, start=False, stop=True)

        def xalloc(s):
            """Allocate step-s PSUM banks for all chunks active at s."""
            act = [c for c in ORDER if s < Ts[c]]
            for c in act:
                banks[c][s] = bankp[c].tile([128, 4, BC], dt.float32,
                                            tag=f"b{c}", name=f"bank{c}")
            return act

        def xmms(s, cs):
            """x-side matmuls for step s for the chunks in cs."""
            # NOTE: start=True clears has_written for the WHOLE bank, so only
            # the first write per bank may use it; later writes to any region
            # use start=False (stores where unwritten, accumulates elsewhere).
            for gi, W in ((0, Kz), (1, Kr), (2, Kh)):
                stop = (s == 0) if gi < 2 else False
                for c in cs:
                    xt = xblks[c][s // 8][:, s % 8, :]
                    nc.tensor.matmul(banks[c][s][:, gi, :], lhsT=W, rhs=xt,
                                     start=(gi == 0), stop=stop)
            if s == 0:
                # seed rh slot with b1h (later steps fold it in via the t1 STT)
                for c in cs:
                    nc.tensor.matmul(banks[c][s][:, 3, :], lhsT=b1h, rhs=ones,
                                     start=False, stop=True)

        def gates1(c, t):
            zrs = gp.tile([100, 2, BC], dt.bfloat16, tag=f"zrs{c}", name=f"zrs{c}",
                           bufs=2)
            nc.scalar.activation(zrs, banks[c][t][0:100, 0:2, :], AF.Sigmoid)
            t1 = gp.tile([100, BC], dt.bfloat16, tag=f"t1{c}", name=f"t1{c}",
                          bufs=2)
            if t == 0:
                nc.vector.tensor_tensor(t1, zrs[:, 1, :], banks[c][t][0:100, 3, :],
                                        OP.mult)
            else:
                # rh already includes b1h (ones-row fold): plain TT
                nc.vector.tensor_tensor(t1, banks[c][t][0:100, 3, :],
                                        zrs[:, 1, :], OP.mult)
            zrs_t[c], t1_t[c] = zrs, t1

        def iacc_group(t):
            for c in ORDER:
                if t >= Ts[c]:
                    continue
                nc.tensor.matmul(banks[c][t][:, 2, :], lhsT=I100, rhs=t1_t[c],
                                 start=False, stop=True)

        def gates2(c, t):
            hh = gp.tile([100, BC], dt.bfloat16, tag=f"hh{c}", name=f"hh{c}",
                          bufs=2)
            nc.scalar.activation(hh, banks[c][t][0:100, 2, :], AF.Tanh)
            stw = stages[c][0:100, t, :]
            if t == 0:
                nc.vector.tensor_tensor(stw, zrs_t[c][:, 0, :], hh, OP.mult)
            else:
                # negw = (z - 1) * h_{t-1}: emitted HERE (not in gates1) so
                # on the DVE FIFO it sits directly ahead of this chunk's u,
                # running in the tanh shadow -- emitting all chunks' negw in
                # gates1 makes chunk 3's u queue behind 3 foreign negws
                ng = gp.tile([100, BC], dt.bfloat16, tag=f"ng{c}", name=f"ng{c}",
                             bufs=2)
                nc.vector.scalar_tensor_tensor(
                    ng, zrs_t[c][:, 0, :], 1.0, stages[c][0:100, t - 1, :],
                    OP.subtract, OP.mult)
                # h_t = u - negw,  u = z*hh  (two back-to-back DVE ops)
                u = gp.tile([100, BC], dt.bfloat16, tag=f"u{c}", name=f"u{c}",
                            bufs=2)
                nc.vector.tensor_tensor(u, zrs_t[c][:, 0, :], hh, OP.mult)
                nc.vector.tensor_tensor(stw, u, ng, OP.subtract)
            del banks[c][t]

        def finish_scan(c):
            tlast = Ts[c] - 1
            lo = gp.tile([100, BC], dt.float32, tag=f"lo{c}", name=f"lo{c}", bufs=1)
            nc.vector.tensor_copy(lo, stages[c][0:100, tlast, :])
            nc.sync.dma_start(out=lastout[c][:, :], in_=lo)

        # --- attention: chunk c processed in GL-step groups, reusing the
        #     retired chunks' PSUM pools; accumulator in SBUF fp32 on GpSimd.
        att = {}

        def att_start(c):
            # two accumulators: even groups accumulate on GpSimd, odd on DVE.
            # One Pool accumulator serializes at ~1.17us/group and saturates
            # the Pool (which shares its SBUF port with the DVE).
            acc = singles.tile([100, GL, BC], mybir.dt.float32, tag=f"accs{c}",
                               name=f"accs{c}")
            nc.vector.memset(acc, 0.0)
            accb = singles.tile([100, GL, BC], mybir.dt.float32, tag=f"accb{c}",
                                name=f"accb{c}")
            nc.vector.memset(accb, 0.0)
            # c1 = A1^T last, precomputed once, replicated into GL step slots
            c1p = bankp[c].tile([128, 4, BC], mybir.dt.float32, tag=f"b{c}",
                                name=f"c1p{c}")
            nc.tensor.matmul(c1p[:, 0, :], lhsT=A1,
                             rhs=stages[c][0:100, Ts[c] - 1, :],
                             start=True, stop=True)
            c1s = singles.tile([100, GL, BC], mybir.dt.bfloat16, tag=f"c1s{c}",
                               name=f"c1s{c}")
            for j in range(GL):
                nc.vector.tensor_copy(c1s[:, j, :], c1p[0:100, 0, :])
            # late chunks rotate over every already-idle pool so the drain
            # pipeline runs as many PSUM banks deep as are free
            partners = {0: [], 1: [0], 2: [1, 0], 3: [0, 1, 2]}[c]
            pools = [(bankp[c], f"b{c}")] + [(bankp[p], f"b{p}") for p in partners]
            att[c] = {"g": 0, "n": Ts[c] // GL, "pools": pools,
                      "acc": acc, "accb": accb, "c1s": c1s, "pend": []}

        def att_done(c):
            return c in att and att[c]["g"] >= att[c]["n"] and not att[c]["pend"]

        def att_stage1(c):
            """A2+c1 matmuls and the sigmoid for the next group."""
            stt = att[c]
            g = stt["g"]
            st4 = stages[c][0:100, GL * g:GL * (g + 1), :]
            pool_, tag_ = stt["pools"][g % len(stt["pools"])]
            sbal = pool_.tile([128, 4, BC], mybir.dt.float32,
                              tag=tag_, name=f"sbal{c}")
            # bank = A2^T st4 + c1 (c1 added on the PE via identity-accumulate)
            nc.tensor.matmul(sbal[:, :, :], lhsT=A2, rhs=st4, start=True, stop=False)
            nc.tensor.matmul(sbal[:, :, :], lhsT=I100, rhs=stt["c1s"],
                             start=False, stop=True)
            g2 = gp.tile([100, GL, BC], mybir.dt.bfloat16, tag=f"g{c}", name=f"g{c}",
                         bufs=2)
            nc.scalar.activation(g2, sbal[0:100, :, :], AF.Sigmoid)
            stt["pend"].append((sbal, g2, st4))
            stt["g"] = g + 1

        def att_stage2(c):
            """alpha matmul + alpha*state + accumulate for the oldest group."""
            stt = att[c]
            stt["r"] = stt.get("r", 0) + 1
            sbal, g2, st4 = stt["pend"].pop(0)
            # alpha (broadcast over partitions) overwrites the same bank
            nc.tensor.matmul(sbal[:, :, :], lhsT=Vr, rhs=g2, start=True, stop=True)
            tmp = gp.tile([100, GL, BC], mybir.dt.bfloat16, tag=f"tmp{c}", name=f"tmp{c}",
                          bufs=2)
            nc.vector.tensor_tensor(tmp, sbal[0:100, :, :], st4, OP.mult)
            if stt["r"] % 2:
                nc.gpsimd.tensor_tensor(stt["acc"], stt["acc"], tmp, OP.add)
            else:
                nc.vector.tensor_tensor(stt["accb"], stt["accb"], tmp, OP.add)
            if stt["g"] >= stt["n"] and not stt["pend"]:
                r2 = gp.tile([100, 2, BC], mybir.dt.float32, tag=f"r2{c}", name=f"r2{c}",
                             bufs=1)
                nc.vector.tensor_tensor(r2, stt["acc"][:, 0:2, :], stt["acc"][:, 2:4, :],
                                        OP.add)
                r2b = gp.tile([100, 2, BC], mybir.dt.float32, tag=f"r2b{c}",
                              name=f"r2b{c}", bufs=1)
                nc.vector.tensor_tensor(r2b, stt["accb"][:, 0:2, :],
                                        stt["accb"][:, 2:4, :], OP.add)
                nc.vector.tensor_tensor(r2, r2, r2b, OP.add)
                osum = gp.tile([100, BC], mybir.dt.float32, tag=f"os{c}", name=f"os{c}",
                               bufs=1)
                nc.vector.tensor_tensor(osum, r2[:, 0, :], r2[:, 1, :], OP.add)
                nc.sync.dma_start(out=outp[c][:, :], in_=osum)

        def att_try_starts(t):
            for c in ORDER:
                if c not in att and t >= Ts[c]:
                    att_start(c)

        def att_pump(budget, depth=2, prefer2=False):
            """budget is in STAGES (1 stage = 1-2 PE matmuls), so scan-time
            pumping injects small slugs of PE work that fit the recurrence's
            idle gaps instead of whole groups that delay the next rec.
            prefer2: retire-first -- used for the mid-step slot so the att
            sigmoid is not queued between the scan sigmoids and tanhs."""
            for c in ORDER:
                if c not in att:
                    continue
                stt = att[c]
                while budget > 0:
                    s2_ready = bool(stt["pend"]) and (
                        prefer2 or len(stt["pend"]) >= depth
                        or stt["g"] >= stt["n"])
                    if s2_ready:
                        att_stage2(c)
                        budget -= 1
                    elif stt["g"] < stt["n"] and len(stt["pend"]) < depth:
                        # stage1 first in the late slot: emitting a retire
                        # (Vr) ahead of the next group's A2 head-of-line
                        # blocks the PE on the sigmoid
                        att_stage1(c)
                        budget -= 1
                    else:
                        break

        # ---------------- emission ----------------
        for c in ORDER:
            issue_xdma(c, 0)
            issue_xdma(c, 1)
        xalloc(0)
        xmms(0, ORDER)

        for t in range(maxT):
            nact = sum(1 for c in range(NCHUNK) if t < Ts[c])
            # rec matmuls first: x(t+1) matmuls carry a WAR wait on the t-1
            # bank's readers, so putting them ahead of the recs couples the
            # longest chunk's chain to the other chunks' tanh reads
            recgroup(t)
            for c in ORDER:
                if t % 8 == 0 and t >= 8:
                    issue_xdma(c, t // 8 + 1)
            if t + 1 < maxT:
                act_next = xalloc(t + 1)
                xmms(t + 1, act_next)
            for c in ORDER:
                if t < Ts[c]:
                    gates1(c, t)
            iacc_group(t)
            bud = {4: 0, 3: 1, 2: 2, 1: 2}.get(nact, 2)
            att_pump(bud - bud // 2, prefer2=True)
            for c in ORDER:
                if t < Ts[c]:
                    gates2(c, t)
                    if t == Ts[c] - 1:
                        finish_scan(c)
            att_try_starts(t)
            att_pump(bud // 2)

        while not all(att_done(c) for c in range(NCHUNK)):
            att_try_starts(10 ** 9)
            att_pump(6, depth=3)

    nc.compile()
    return nc


def _prep_weights(kernel_w, rec_kernel, bias_, A1_w, A2_w, v):
    b0, b1 = bias_[0], bias_[1]
    wall = np.zeros((128, 12, 128), np.float32)
    wall[:E, 0, :U] = -kernel_w[:, :U]
    wall[100, 0, :U] = -40.0
    wall[101, 0, :U] = -(b0[:U] + b1[:U])
    wall[:E, 1, :U] = kernel_w[:, U:2 * U]
    wall[101, 1, :U] = b0[U:2 * U] + b1[U:2 * U]
    wall[:E, 2, :U] = kernel_w[:, 2 * U:]
    wall[101, 2, :U] = b0[2 * U:]
    wall[:U, 3, :U] = -rec_kernel[:, :U]
    wall[:U, 4, :U] = rec_kernel[:, U:2 * U]
    wall[:U, 5, :U] = rec_kernel[:, 2 * U:]
    wall[100, 5, :U] = b1[2 * U:]
    wall[:U, 6, :U] = A1_w
    wall[:U, 7, :U] = A2_w
    wall[:U, 8, :U] = np.broadcast_to(v[0][:, None], (U, U))
    wall[:U, 9, :U] = np.eye(U, dtype=np.float32)
    wall[0, 10, :U] = b1[2 * U:]
    wall[:U, 11, 0] = b1[2 * U:]
    return {"wall": wall.astype(bf16),
            "wones": np.ones((1, T, BC), bf16)}


def kernel(session_hidden, mask, kernel, rec_kernel, bias, A1_w, A2_w, v):
    session_hidden = np.asarray(session_hidden, np.float32)
    mask = np.asarray(mask, np.float32)
    kernel_w = np.asarray(kernel, np.float32)
    rec_kernel = np.asarray(rec_kernel, np.float32)
    bias_ = np.asarray(bias, np.float32)
    A1_w = np.asarray(A1_w, np.float32)
    A2_w = np.asarray(A2_w, np.float32)
    v = np.asarray(v, np.float32)

    lengths = mask.sum(1).astype(np.int64)  # in [1, T]
    order = np.argsort(lengths, kind="stable")
    # deal round-robin: sorted rank i -> core i%8, slot i//8
    slot = np.arange(B) // NCORES
    core = np.arange(B) % NCORES
    perm = np.empty(B, np.int64)
    perm[core * PERCORE + slot] = order  # arranged[core*512+slot] = orig row
    lens_a = lengths[perm]
    lens_sorted = lengths[order]
    Ts = tuple(_ceil8(lens_sorted[NCORES * BC * (c + 1) - 1])
               for c in range(NCHUNK))

    key = Ts
    if key not in _CACHE:
        _CACHE[key] = _build(Ts)
    nc = _CACHE[key]
    _CACHE["nc"] = nc

    w = _prep_weights(kernel_w, rec_kernel, bias_, A1_w, A2_w, v)

    x_a = session_hidden[perm].reshape(NCORES, NCHUNK, BC, T, E)
    m_a = mask[perm].reshape(NCORES, NCHUNK, BC, T)
    in_maps = []
    for k in range(NCORES):
        im = dict(w)
        for c in range(NCHUNK):
            Tc = Ts[c]
            xc = np.zeros((128, Tc, BC), np.float32)
            xc[:E] = x_a[k, c, :, :Tc, :].transpose(2, 1, 0)
            xc[100] = 1.0 - m_a[k, c, :, :Tc].transpose(1, 0)
            xc[101] = 1.0
            im[f"xc{c}"] = xc.astype(bf16)
        in_maps.append(im)

    _CACHE["in_maps"] = in_maps
    res = bass_utils.run_bass_kernel_spmd(nc, in_maps, core_ids=list(range(NCORES)))

    out_dev = np.zeros((B, U), np.float32)
    last = np.zeros((B, U), np.float32)
    for k in range(NCORES):
        r = res.results[k]
        for c in range(NCHUNK):
            sl_ = slice(k * PERCORE + c * BC, k * PERCORE + (c + 1) * BC)
            out_dev[sl_] = np.asarray(r[f"outp{c}"]).T.astype(np.float32)
            last[sl_] = np.asarray(r[f"lastc{c}"]).T.astype(np.float32)

    # host correction: device ran steps [0, T_c) with the A1*last term for all t.
    # truth: masked t in [len, T) contribute sigmoid(A2^T last)@v * last.
    Tc_a = np.tile(np.repeat(np.asarray(Ts, np.float32), BC), NCORES)
    sl_ = last @ A2_w
    c_ = last @ A1_w
    sig = lambda a: 1.0 / (1.0 + np.exp(-a))
    a1 = sig(sl_ + c_) @ v[0]
    a0 = sig(sl_) @ v[0]
    lf = lens_a.astype(np.float32)
    out_a = out_dev - (Tc_a - lf)[:, None] * a1[:, None] * last \
        + (T - lf)[:, None] * a0[:, None] * last

    out = np.empty((B, U), np.float32)
    out[perm] = out_a
    _CACHE["debug"] = dict(out_dev=out_dev, last=last, perm=perm, Ts=Ts,
                           lens_a=lens_a, out_a=out_a)
    return out.astype(np.float32)


# revision 36
# speedup vs baseline: 1.0074x; 1.0074x over previous
"""Trainium2 Bass kernel for nn_LocalEncoder (masked GRU + attention pooling).

Strategy (v4, 735us vs 861us baseline; wall ~= 200 steps x per-step chain):
- Data-parallel over batch: 8 cores x 512 rows. Rows are length-sorted and
  dealt round-robin so every core gets an identical length profile, then
  split into 4 chunks of 128 (short->long). Chunk c only scans T_c steps
  (T_c = max length in chunk, uniform across cores) - ~25% less work.
- Feature-major [U partitions, batch free]. All matmuls bf16 -> fp32 PSUM.
- Scan: per chunk-step one PSUM bank holds [z|r|xh|rh]. Emission order is
  latency-first: rec matmuls for step t (longest chunk first) BEFORE the
  x-side matmuls for t+1 (x MMs carry a WAR on the t-1 bank's readers, so
  putting them first couples chunk 3's chain to the other chunks). The
  per-step chain (~2.7us for the longest chunk) is
    rec(z,r,h) -> sigmoid(z|r) -> t1 -> iacc -> tanh -> u,stw -> rec(t+1)
  with the z-blend split as  h_t = u - negw,  u = z*hh (DVE, on-chain),
  negw = (z-1)*h_{t-1} (DVE, emitted just before u so it runs in the tanh
  shadow; emitting all chunks' negw in gates1 queues 3 foreign negws ahead
  of chunk 3's u). b1h is folded into the rh matmul via a constant-ones
  row 100 in the stage (t1 is then a plain TT, not STT). Trailing-padding
  mask folded via -40 * (1-m) row into the z-gate (freezes h exactly).
- All weights live in one pre-padded [128,12,128] "wall" tensor: one DMA,
  full 128-col stationaries (FWL-eligible), no on-device memset/pad.
- All state stays in SBUF: stage[c] = [101, T_c, BC] bf16 (row 100 = 1).
- Attention: sigmoid(A1*last + A2*state_t) with the A1 term applied for
  ALL t; host subtracts the closed-form correction for masked steps and
  adds the contribution of steps beyond T_c. 4-step groups (1 PSUM bank,
  same slot shape as the scan banks), c1 added via a PE identity-
  accumulate, alpha written back over the same bank by the Vr matmul,
  alpha*state on DVE, accumulation ALTERNATING GpSimd/DVE (one Pool
  accumulator serializes at ~1.17us/group and the Pool shares its SBUF
  port with the DVE). Groups are software-pipelined in two stages
  (A2+c1+sigmoid | Vr+mult+acc), emitted stage1-first so the PE never
  head-of-line blocks on a Vr waiting for its sigmoid; pumped into engine
  slack during the scan and drained 3-deep over all 4 PSUM pools after.
"""
import sys
sys.path.insert(0, "/opt/trn_rl_repo")
from contextlib import ExitStack

import numpy as np
import ml_dtypes

import concourse.bass as bass
import concourse.bacc as bacc
import concourse.tile as tile
from concourse import mybir
from concourse import bass_utils

bf16 = ml_dtypes.bfloat16
AF = mybir.ActivationFunctionType
OP = mybir.AluOpType

B, T, E, U = 4096, 200, 100, 100
NCORES = 8
NCHUNK = 4
BC = 128
PERCORE = NCHUNK * BC
GL = 4  # attention group length (steps per group); 1 PSUM bank per group

_CACHE = {}


def _ceil8(x):
    return min(((int(x) + 7) // 8) * 8, T)


def _build(Ts):
    """Ts: per-chunk step counts (uniform across cores)."""
    nc = bacc.Bacc()
    dt = mybir.dt

    xcs = [nc.dram_tensor(f"xc{c}", [128, Ts[c], BC], dt.bfloat16,
                          kind="ExternalInput") for c in range(NCHUNK)]
    # all stationary weights pre-padded to [128,128] host-side, one DMA
    wall_d = nc.dram_tensor("wall", [128, 12, 128], dt.bfloat16,
                            kind="ExternalInput")
    ones_d = nc.dram_tensor("wones", [1, T, BC], dt.bfloat16,
                            kind="ExternalInput")
    lastout = [nc.dram_tensor(f"lastc{c}", [U, BC], dt.float32,
                              kind="ExternalOutput") for c in range(NCHUNK)]
    outp = [nc.dram_tensor(f"outp{c}", [U, BC], dt.float32,
                           kind="ExternalOutput") for c in range(NCHUNK)]

    maxT = max(Ts)
    ORDER = list(range(NCHUNK - 1, -1, -1))  # longest chunk first

    with tile.TileContext(nc) as tc, ExitStack() as octx:
        singles = octx.enter_context(tc.tile_pool(name="singles", bufs=1))
        xpool = octx.enter_context(tc.tile_pool(name="xpool", bufs=2))
        gp = octx.enter_context(tc.tile_pool(name="gp", bufs=3))
        bankp = [octx.enter_context(
            tc.tile_pool(name=f"bankp{c}", bufs=2, space="PSUM"))
            for c in range(NCHUNK)]

        wall = singles.tile([128, 12, 128], dt.bfloat16, tag="wall", name="wall")
        nc.sync.dma_start(out=wall, in_=wall_d[:, :, :])
        Kz, Kr, Kh = wall[:, 0, :], wall[:, 1, :], wall[:, 2, :]
        Rz, Rr, Rh = wall[0:101, 3, :], wall[0:101, 4, :], wall[0:101, 5, :]
        A1, A2 = wall[0:U, 6, :], wall[0:U, 7, :]
        Vr, I100 = wall[0:U, 8, :], wall[0:U, 9, :]
        b1h = wall[0:1, 10, :]
        b1hT = wall[0:U, 11, 0:1]
        ones = singles.tile([1, BC], dt.bfloat16, tag="ones")
        nc.vector.memset(ones, 1.0)

        stages = []
        for c in range(NCHUNK):
            # row 100 is a constant-ones row: the rec rhs is [101,BC] so the
            # Rh bias row folds b1h into the rh matmul (t1 becomes a plain TT)
            st = singles.tile([101, Ts[c], BC], dt.bfloat16, tag=f"stage{c}",
                              name=f"stage{c}")
            nc.sync.dma_start(out=st[100:101, :, :], in_=ones_d[:, 0:Ts[c], :])
            stages.append(st)

        xblks = [dict() for _ in range(NCHUNK)]
        banks = [dict() for _ in range(NCHUNK)]
        zrs_t = [None] * NCHUNK
        t1_t = [None] * NCHUNK
        negw_t = [None] * NCHUNK

        def issue_xdma(c, k):
            if k * 8 >= Ts[c]:
                return
            xt = xpool.tile([128, 8, BC], dt.bfloat16, tag=f"x{c}", name=f"xb{c}")
            nc.sync.dma_start(out=xt, in_=xcs[c][:, k * 8:(k + 1) * 8, :])
            xblks[c][k] = xt

        def recgroup(t):
            for c in ORDER:
                if t < 1 or t >= Ts[c]:
                    continue
                h = stages[c][0:101, t - 1, :]
                bk = banks[c][t]
                nc.tensor.matmul(bk[:, 0, :], lhsT=Rz, rhs=h, start=False, stop=True)
                nc.tensor.matmul(bk[:, 1, :], lhsT=Rr, rhs=h, start=False, stop=True)
                nc.tensor.matmul(bk[:, 3, :], lhsT=Rh, rhs=h, start=False, stop=True)

        def xalloc(s):
            """Allocate step-s PSUM banks for all chunks active at s."""
            act = [c for c in ORDER if s < Ts[c]]
            for c in act:
                banks[c][s] = bankp[c].tile([128, 4, BC], dt.float32,
                                            tag=f"b{c}", name=f"bank{c}")
            return act

        def xmms(s, cs):
            """x-side matmuls for step s for the chunks in cs."""
            # NOTE: start=True clears has_written for the WHOLE bank, so only
            # the first write per bank may use it; later writes to any region
            # use start=False (stores where unwritten, accumulates elsewhere).
            for gi, W in ((0, Kz), (1, Kr), (2, Kh)):
                stop = (s == 0) if gi < 2 else False
                for c in cs:
                    xt = xblks[c][s // 8][:, s % 8, :]
                    nc.tensor.matmul(banks[c][s][:, gi, :], lhsT=W, rhs=xt,
                                     start=(gi == 0), stop=stop)
            if s == 0:
                # seed rh slot with b1h (later steps fold it in via the t1 STT)
                for c in cs:
                    nc.tensor.matmul(banks[c][s][:, 3, :], lhsT=b1h, rhs=ones,
                                     start=False, stop=True)

        def gates1(c, t):
            zrs = gp.tile([100, 2, BC], dt.bfloat16, tag=f"zrs{c}", name=f"zrs{c}",
                           bufs=2)
            nc.scalar.activation(zrs, banks[c][t][0:100, 0:2, :], AF.Sigmoid)
            t1 = gp.tile([100, BC], dt.bfloat16, tag=f"t1{c}", name=f"t1{c}",
                          bufs=2)
            if t == 0:
                nc.vector.tensor_tensor(t1, zrs[:, 1, :], banks[c][t][0:100, 3, :],
                                        OP.mult)
            else:
                # rh already includes b1h (ones-row fold): plain TT
                nc.vector.tensor_tensor(t1, banks[c][t][0:100, 3, :],
                                        zrs[:, 1, :], OP.mult)
            zrs_t[c], t1_t[c] = zrs, t1

        def iacc_group(t):
            for c in ORDER:
                if t >= Ts[c]:
                    continue
                nc.tensor.matmul(banks[c][t][:, 2, :], lhsT=I100, rhs=t1_t[c],
                                 start=False, stop=True)

        def gates2(c, t):
            hh = gp.tile([100, BC], dt.bfloat16, tag=f"hh{c}", name=f"hh{c}",
                          bufs=2)
            nc.scalar.activation(hh, banks[c][t][0:100, 2, :], AF.Tanh)
            stw = stages[c][0:100, t, :]
            if t == 0:
                nc.vector.tensor_tensor(stw, zrs_t[c][:, 0, :], hh, OP.mult)
            else:
                # negw = (z - 1) * h_{t-1}: emitted HERE (not in gates1) so
                # on the DVE FIFO it sits directly ahead of this chunk's u,
                # running in the tanh shadow -- emitting all chunks' negw in
                # gates1 makes chunk 3's u queue behind 3 foreign negws
                ng = gp.tile([100, BC], dt.bfloat16, tag=f"ng{c}", name=f"ng{c}",
                             bufs=2)
                nc.vector.scalar_tensor_tensor(
                    ng, zrs_t[c][:, 0, :], 1.0, stages[c][0:100, t - 1, :],
                    OP.subtract, OP.mult)
                # h_t = u - negw,  u = z*hh  (two back-to-back DVE ops)
                u = gp.tile([100, BC], dt.bfloat16, tag=f"u{c}", name=f"u{c}",
                            bufs=2)
                nc.vector.tensor_tensor(u, zrs_t[c][:, 0, :], hh, OP.mult)
                nc.vector.tensor_tensor(stw, u, ng, OP.subtract)
            del banks[c][t]

        def finish_scan(c):
            tlast = Ts[c] - 1
            lo = gp.tile([100, BC], dt.float32, tag=f"lo{c}", name=f"lo{c}", bufs=1)
            nc.vector.tensor_copy(lo, stages[c][0:100, tlast, :])
            nc.sync.dma_start(out=lastout[c][:, :], in_=lo)

        # --- attention: chunk c processed in GL-step groups, reusing the
        #     retired chunks' PSUM pools; accumulator in SBUF fp32 on GpSimd.
        att = {}

        def att_start(c):
            # two accumulators: even groups accumulate on GpSimd, odd on DVE.
            # One Pool accumulator serializes at ~1.17us/group and saturates
            # the Pool (which shares its SBUF port with the DVE).
            acc = singles.tile([100, GL, BC], mybir.dt.float32, tag=f"accs{c}",
                               name=f"accs{c}")
            nc.vector.memset(acc, 0.0)
            accb = singles.tile([100, GL, BC], mybir.dt.float32, tag=f"accb{c}",
                                name=f"accb{c}")
            nc.vector.memset(accb, 0.0)
            # c1 = A1^T last, precomputed once, replicated into GL step slots
            c1p = bankp[c].tile([128, 4, BC], mybir.dt.float32, tag=f"b{c}",
                                name=f"c1p{c}")
            nc.tensor.matmul(c1p[:, 0, :], lhsT=A1,
                             rhs=stages[c][0:100, Ts[c] - 1, :],
                             start=True, stop=True)
            c1s = singles.tile([100, GL, BC], mybir.dt.bfloat16, tag=f"c1s{c}",
                               name=f"c1s{c}")
            for j in range(GL):
                nc.vector.tensor_copy(c1s[:, j, :], c1p[0:100, 0, :])
            # late chunks rotate over every already-idle pool so the drain
            # pipeline runs as many PSUM banks deep as are free
            partners = {0: [], 1: [0], 2: [1, 0], 3: [0, 1, 2]}[c]
            pools = [(bankp[c], f"b{c}")] + [(bankp[p], f"b{p}") for p in partners]
            att[c] = {"g": 0, "n": Ts[c] // GL, "pools": pools,
                      "acc": acc, "accb": accb, "c1s": c1s, "pend": []}

        def att_done(c):
            return c in att and att[c]["g"] >= att[c]["n"] and not att[c]["pend"]

        def att_stage1(c):
            """A2+c1 matmuls and the sigmoid for the next group."""
            stt = att[c]
            g = stt["g"]
            st4 = stages[c][0:100, GL * g:GL * (g + 1), :]
            pool_, tag_ = stt["pools"][g % len(stt["pools"])]
            sbal = pool_.tile([128, 4, BC], mybir.dt.float32,
                              tag=tag_, name=f"sbal{c}")
            # bank = A2^T st4 + c1 (c1 added on the PE via identity-accumulate)
            nc.tensor.matmul(sbal[:, :, :], lhsT=A2, rhs=st4, start=True, stop=False)
            nc.tensor.matmul(sbal[:, :, :], lhsT=I100, rhs=stt["c1s"],
                             start=False, stop=True)
            g2 = gp.tile([100, GL, BC], mybir.dt.bfloat16, tag=f"g{c}", name=f"g{c}",
                         bufs=2)
            nc.scalar.activation(g2, sbal[0:100, :, :], AF.Sigmoid)
            stt["pend"].append((sbal, g2, st4))
            stt["g"] = g + 1

        def att_stage2(c):
            """alpha matmul + alpha*state + accumulate for the oldest group."""
            stt = att[c]
            stt["r"] = stt.get("r", 0) + 1
            sbal, g2, st4 = stt["pend"].pop(0)
            # alpha (broadcast over partitions) overwrites the same bank
            nc.tensor.matmul(sbal[:, :, :], lhsT=Vr, rhs=g2, start=True, stop=True)
            tmp = gp.tile([100, GL, BC], mybir.dt.bfloat16, tag=f"tmp{c}", name=f"tmp{c}",
                          bufs=2)
            nc.vector.tensor_tensor(tmp, sbal[0:100, :, :], st4, OP.mult)
            if stt["r"] % 2:
                nc.gpsimd.tensor_tensor(stt["acc"], stt["acc"], tmp, OP.add)
            else:
                nc.vector.tensor_tensor(stt["accb"], stt["accb"], tmp, OP.add)
            if stt["g"] >= stt["n"] and not stt["pend"]:
                r2 = gp.tile([100, 2, BC], mybir.dt.float32, tag=f"r2{c}", name=f"r2{c}",
                             bufs=1)
                nc.vector.tensor_tensor(r2, stt["acc"][:, 0:2, :], stt["acc"][:, 2:4, :],
                                        OP.add)
                r2b = gp.tile([100, 2, BC], mybir.dt.float32, tag=f"r2b{c}",
                              name=f"r2b{c}", bufs=1)
                nc.vector.tensor_tensor(r2b, stt["accb"][:, 0:2, :],
                                        stt["accb"][:, 2:4, :], OP.add)
                nc.vector.tensor_tensor(r2, r2, r2b, OP.add)
                osum = gp.tile([100, BC], mybir.dt.float32, tag=f"os{c}", name=f"os{c}",
                               bufs=1)
                nc.vector.tensor_tensor(osum, r2[:, 0, :], r2[:, 1, :], OP.add)
                nc.sync.dma_start(out=outp[c][:, :], in_=osum)

        def att_try_starts(t):
            for c in ORDER:
                if c not in att and t >= Ts[c]:
                    att_start(c)

        def att_pump(budget, depth=2):
            """budget is in STAGES (1 stage = 1-2 PE matmuls), so scan-time
            pumping injects small slugs of PE work that fit the recurrence's
            idle gaps instead of whole groups that delay the next rec."""
            for c in ORDER:
                if c not in att:
                    continue
                stt = att[c]
                while budget > 0:
                    # stage1 first: emitting a retire (Vr) ahead of the next
                    # group's A2 head-of-line blocks the PE on the sigmoid
                    if stt["g"] < stt["n"] and len(stt["pend"]) < depth:
                        att_stage1(c)
                        budget -= 1
                    elif stt["pend"]:
                        att_stage2(c)
                        budget -= 1
                    else:
                        break

        # ---------------- emission ----------------
        for c in ORDER:
            issue_xdma(c, 0)
            issue_xdma(c, 1)
        xalloc(0)
        xmms(0, ORDER)

        for t in range(maxT):
            nact = sum(1 for c in range(NCHUNK) if t < Ts[c])
            # rec matmuls first: x(t+1) matmuls carry a WAR wait on the t-1
            # bank's readers, so putting them ahead of the recs couples the
            # longest chunk's chain to the other chunks' tanh reads
            recgroup(t)
            for c in ORDER:
                if t % 8 == 0 and t >= 8:
                    issue_xdma(c, t // 8 + 1)
            if t + 1 < maxT:
                act_next = xalloc(t + 1)
                xmms(t + 1, act_next)
            for c in ORDER:
                if t < Ts[c]:
                    gates1(c, t)
            iacc_group(t)
            for c in ORDER:
                if t < Ts[c]:
                    gates2(c, t)
                    if t == Ts[c] - 1:
                        finish_scan(c)
            att_try_starts(t)
            att_pump({4: 0, 3: 1, 2: 2, 1: 2}.get(nact, 2))

        while not all(att_done(c) for c in range(NCHUNK)):
            att_try_starts(10 ** 9)
            att_pump(6, depth=3)

    nc.compile()
    return nc


def _prep_weights(kernel_w, rec_kernel, bias_, A1_w, A2_w, v):
    b0, b1 = bias_[0], bias_[1]
    wall = np.zeros((128, 12, 128), np.float32)
    wall[:E, 0, :U] = -kernel_w[:, :U]
    wall[100, 0, :U] = -40.0
    wall[101, 0, :U] = -(b0[:U] + b1[:U])
    wall[:E, 1, :U] = kernel_w[:, U:2 * U]
    wall[101, 1, :U] = b0[U:2 * U] + b1[U:2 * U]
    wall[:E, 2, :U] = kernel_w[:, 2 * U:]
    wall[101, 2, :U] = b0[2 * U:]
    wall[:U, 3, :U] = -rec_kernel[:, :U]
    wall[:U, 4, :U] = rec_kernel[:, U:2 * U]
    wall[:U, 5, :U] = rec_kernel[:, 2 * U:]
    wall[100, 5, :U] = b1[2 * U:]
    wall[:U, 6, :U] = A1_w
    wall[:U, 7, :U] = A2_w
    wall[:U, 8, :U] = np.broadcast_to(v[0][:, None], (U, U))
    wall[:U, 9, :U] = np.eye(U, dtype=np.float32)
    wall[0, 10, :U] = b1[2 * U:]
    wall[:U, 11, 0] = b1[2 * U:]
    return {"wall": wall.astype(bf16),
            "wones": np.ones((1, T, BC), bf16)}


def kernel(session_hidden, mask, kernel, rec_kernel, bias, A1_w, A2_w, v):
    session_hidden = np.asarray(session_hidden, np.float32)
    mask = np.asarray(mask, np.float32)
    kernel_w = np.asarray(kernel, np.float32)
    rec_kernel = np.asarray(rec_kernel, np.float32)
    bias_ = np.asarray(bias, np.float32)
    A1_w = np.asarray(A1_w, np.float32)
    A2_w = np.asarray(A2_w, np.float32)
    v = np.asarray(v, np.float32)

    lengths = mask.sum(1).astype(np.int64)  # in [1, T]
    order = np.argsort(lengths, kind="stable")
    # deal round-robin: sorted rank i -> core i%8, slot i//8
    slot = np.arange(B) // NCORES
    core = np.arange(B) % NCORES
    perm = np.empty(B, np.int64)
    perm[core * PERCORE + slot] = order  # arranged[core*512+slot] = orig row
    lens_a = lengths[perm]
    lens_sorted = lengths[order]
    Ts = tuple(_ceil8(lens_sorted[NCORES * BC * (c + 1) - 1])
               for c in range(NCHUNK))

    key = Ts
    if key not in _CACHE:
        _CACHE[key] = _build(Ts)
    nc = _CACHE[key]
    _CACHE["nc"] = nc

    w = _prep_weights(kernel_w, rec_kernel, bias_, A1_w, A2_w, v)

    x_a = session_hidden[perm].reshape(NCORES, NCHUNK, BC, T, E)
    m_a = mask[perm].reshape(NCORES, NCHUNK, BC, T)
    in_maps = []
    for k in range(NCORES):
        im = dict(w)
        for c in range(NCHUNK):
            Tc = Ts[c]
            xc = np.zeros((128, Tc, BC), np.float32)
            xc[:E] = x_a[k, c, :, :Tc, :].transpose(2, 1, 0)
            xc[100] = 1.0 - m_a[k, c, :, :Tc].transpose(1, 0)
            xc[101] = 1.0
            im[f"xc{c}"] = xc.astype(bf16)
        in_maps.append(im)

    _CACHE["in_maps"] = in_maps
    res = bass_utils.run_bass_kernel_spmd(nc, in_maps, core_ids=list(range(NCORES)))

    out_dev = np.zeros((B, U), np.float32)
    last = np.zeros((B, U), np.float32)
    for k in range(NCORES):
        r = res.results[k]
        for c in range(NCHUNK):
            sl_ = slice(k * PERCORE + c * BC, k * PERCORE + (c + 1) * BC)
            out_dev[sl_] = np.asarray(r[f"outp{c}"]).T.astype(np.float32)
            last[sl_] = np.asarray(r[f"lastc{c}"]).T.astype(np.float32)

    # host correction: device ran steps [0, T_c) with the A1*last term for all t.
    # truth: masked t in [len, T) contribute sigmoid(A2^T last)@v * last.
    Tc_a = np.tile(np.repeat(np.asarray(Ts, np.float32), BC), NCORES)
    sl_ = last @ A2_w
    c_ = last @ A1_w
    sig = lambda a: 1.0 / (1.0 + np.exp(-a))
    a1 = sig(sl_ + c_) @ v[0]
    a0 = sig(sl_) @ v[0]
    lf = lens_a.astype(np.float32)
    out_a = out_dev - (Tc_a - lf)[:, None] * a1[:, None] * last \
        + (T - lf)[:, None] * a0[:, None] * last

    out = np.empty((B, U), np.float32)
    out[perm] = out_a
    _CACHE["debug"] = dict(out_dev=out_dev, last=last, perm=perm, Ts=Ts,
                           lens_a=lens_a, out_a=out_a)
    return out.astype(np.float32)


# revision 37
# speedup vs baseline: 1.0086x; 1.0012x over previous
"""Trainium2 Bass kernel for nn_LocalEncoder (masked GRU + attention pooling).

Strategy (v4, 735us vs 861us baseline; wall ~= 200 steps x per-step chain):
- Data-parallel over batch: 8 cores x 512 rows. Rows are length-sorted and
  dealt round-robin so every core gets an identical length profile, then
  split into 4 chunks of 128 (short->long). Chunk c only scans T_c steps
  (T_c = max length in chunk, uniform across cores) - ~25% less work.
- Feature-major [U partitions, batch free]. All matmuls bf16 -> fp32 PSUM.
- Scan: per chunk-step one PSUM bank holds [z|r|xh|rh]. Emission order is
  latency-first: rec matmuls for step t (longest chunk first) BEFORE the
  x-side matmuls for t+1 (x MMs carry a WAR on the t-1 bank's readers, so
  putting them first couples chunk 3's chain to the other chunks). The
  per-step chain (~2.7us for the longest chunk) is
    rec(z,r,h) -> sigmoid(z|r) -> t1 -> iacc -> tanh -> u,stw -> rec(t+1)
  with the z-blend split as  h_t = u - negw,  u = z*hh (DVE, on-chain),
  negw = (z-1)*h_{t-1} (DVE, emitted just before u so it runs in the tanh
  shadow; emitting all chunks' negw in gates1 queues 3 foreign negws ahead
  of chunk 3's u). b1h is folded into the rh matmul via a constant-ones
  row 100 in the stage (t1 is then a plain TT, not STT). Trailing-padding
  mask folded via -40 * (1-m) row into the z-gate (freezes h exactly).
- All weights live in one pre-padded [128,12,128] "wall" tensor: one DMA,
  full 128-col stationaries (FWL-eligible), no on-device memset/pad.
- All state stays in SBUF: stage[c] = [101, T_c, BC] bf16 (row 100 = 1).
- Attention: sigmoid(A1*last + A2*state_t) with the A1 term applied for
  ALL t; host subtracts the closed-form correction for masked steps and
  adds the contribution of steps beyond T_c. 4-step groups (1 PSUM bank,
  same slot shape as the scan banks), c1 added via a PE identity-
  accumulate, alpha written back over the same bank by the Vr matmul,
  alpha*state on DVE, accumulation ALTERNATING GpSimd/DVE (one Pool
  accumulator serializes at ~1.17us/group and the Pool shares its SBUF
  port with the DVE). Groups are software-pipelined in two stages
  (A2+c1+sigmoid | Vr+mult+acc), emitted stage1-first so the PE never
  head-of-line blocks on a Vr waiting for its sigmoid; pumped into engine
  slack during the scan and drained 3-deep over all 4 PSUM pools after.
"""
import sys
sys.path.insert(0, "/opt/trn_rl_repo")
from contextlib import ExitStack

import numpy as np
import ml_dtypes

import concourse.bass as bass
import concourse.bacc as bacc
import concourse.tile as tile
from concourse import mybir
from concourse import bass_utils

bf16 = ml_dtypes.bfloat16
AF = mybir.ActivationFunctionType
OP = mybir.AluOpType

B, T, E, U = 4096, 200, 100, 100
NCORES = 8
NCHUNK = 4
BC = 128
PERCORE = NCHUNK * BC
GL = 4  # attention group length (steps per group); 1 PSUM bank per group

_CACHE = {}


def _ceil8(x):
    return min(((int(x) + 7) // 8) * 8, T)


def _build(Ts):
    """Ts: per-chunk step counts (uniform across cores)."""
    nc = bacc.Bacc()
    dt = mybir.dt

    xcs = [nc.dram_tensor(f"xc{c}", [128, Ts[c], BC], dt.bfloat16,
                          kind="ExternalInput") for c in range(NCHUNK)]
    # all stationary weights pre-padded to [128,128] host-side, one DMA
    wall_d = nc.dram_tensor("wall", [128, 12, 128], dt.bfloat16,
                            kind="ExternalInput")
    ones_d = nc.dram_tensor("wones", [1, T, BC], dt.bfloat16,
                            kind="ExternalInput")
    lastout = [nc.dram_tensor(f"lastc{c}", [U, BC], dt.float32,
                              kind="ExternalOutput") for c in range(NCHUNK)]
    outp = [nc.dram_tensor(f"outp{c}", [U, BC], dt.float32,
                           kind="ExternalOutput") for c in range(NCHUNK)]

    maxT = max(Ts)
    ORDER = list(range(NCHUNK - 1, -1, -1))  # longest chunk first

    with tile.TileContext(nc) as tc, ExitStack() as octx:
        singles = octx.enter_context(tc.tile_pool(name="singles", bufs=1))
        xpool = octx.enter_context(tc.tile_pool(name="xpool", bufs=2))
        gp = octx.enter_context(tc.tile_pool(name="gp", bufs=3))
        bankp = [octx.enter_context(
            tc.tile_pool(name=f"bankp{c}", bufs=2, space="PSUM"))
            for c in range(NCHUNK)]

        wall = singles.tile([128, 12, 128], dt.bfloat16, tag="wall", name="wall")
        nc.sync.dma_start(out=wall, in_=wall_d[:, :, :])
        Kz, Kr, Kh = wall[:, 0, :], wall[:, 1, :], wall[:, 2, :]
        Rz, Rr, Rh = wall[0:101, 3, :], wall[0:101, 4, :], wall[0:101, 5, :]
        A1, A2 = wall[0:U, 6, :], wall[0:U, 7, :]
        Vr, I100 = wall[0:U, 8, :], wall[0:U, 9, :]
        b1h = wall[0:1, 10, :]
        b1hT = wall[0:U, 11, 0:1]
        ones = singles.tile([1, BC], dt.bfloat16, tag="ones")
        nc.vector.memset(ones, 1.0)

        stages = []
        for c in range(NCHUNK):
            # row 100 is a constant-ones row: the rec rhs is [101,BC] so the
            # Rh bias row folds b1h into the rh matmul (t1 becomes a plain TT)
            st = singles.tile([101, Ts[c], BC], dt.bfloat16, tag=f"stage{c}",
                              name=f"stage{c}")
            nc.sync.dma_start(out=st[100:101, :, :], in_=ones_d[:, 0:Ts[c], :])
            stages.append(st)

        xblks = [dict() for _ in range(NCHUNK)]
        banks = [dict() for _ in range(NCHUNK)]
        zrs_t = [None] * NCHUNK
        t1_t = [None] * NCHUNK
        negw_t = [None] * NCHUNK

        def issue_xdma(c, k):
            if k * 8 >= Ts[c]:
                return
            xt = xpool.tile([128, 8, BC], dt.bfloat16, tag=f"x{c}", name=f"xb{c}")
            nc.sync.dma_start(out=xt, in_=xcs[c][:, k * 8:(k + 1) * 8, :])
            xblks[c][k] = xt

        def recgroup(t):
            for c in ORDER:
                if t < 1 or t >= Ts[c]:
                    continue
                h = stages[c][0:101, t - 1, :]
                bk = banks[c][t]
                nc.tensor.matmul(bk[:, 0, :], lhsT=Rz, rhs=h, start=False, stop=True)
                nc.tensor.matmul(bk[:, 1, :], lhsT=Rr, rhs=h, start=False, stop=True)
                nc.tensor.matmul(bk[:, 3, :], lhsT=Rh, rhs=h, start=False, stop=True)

        def xalloc(s):
            """Allocate step-s PSUM banks for all chunks active at s."""
            act = [c for c in ORDER if s < Ts[c]]
            for c in act:
                banks[c][s] = bankp[c].tile([128, 4, BC], dt.float32,
                                            tag=f"b{c}", name=f"bank{c}")
            return act

        def xmms(s, cs):
            """x-side matmuls for step s for the chunks in cs."""
            # NOTE: start=True clears has_written for the WHOLE bank, so only
            # the first write per bank may use it; later writes to any region
            # use start=False (stores where unwritten, accumulates elsewhere).
            for gi, W in ((0, Kz), (1, Kr), (2, Kh)):
                stop = (s == 0) if gi < 2 else False
                for c in cs:
                    xt = xblks[c][s // 8][:, s % 8, :]
                    nc.tensor.matmul(banks[c][s][:, gi, :], lhsT=W, rhs=xt,
                                     start=(gi == 0), stop=stop)
            if s == 0:
                # seed rh slot with b1h (later steps fold it in via the t1 STT)
                for c in cs:
                    nc.tensor.matmul(banks[c][s][:, 3, :], lhsT=b1h, rhs=ones,
                                     start=False, stop=True)

        def gates1(c, t):
            zrs = gp.tile([100, 2, BC], dt.bfloat16, tag=f"zrs{c}", name=f"zrs{c}",
                           bufs=2)
            nc.scalar.activation(zrs, banks[c][t][0:100, 0:2, :], AF.Sigmoid)
            t1 = gp.tile([100, BC], dt.bfloat16, tag=f"t1{c}", name=f"t1{c}",
                          bufs=2)
            if t == 0:
                nc.vector.tensor_tensor(t1, zrs[:, 1, :], banks[c][t][0:100, 3, :],
                                        OP.mult)
            else:
                # rh already includes b1h (ones-row fold): plain TT
                nc.vector.tensor_tensor(t1, banks[c][t][0:100, 3, :],
                                        zrs[:, 1, :], OP.mult)
            zrs_t[c], t1_t[c] = zrs, t1

        def iacc_group(t):
            for c in ORDER:
                if t >= Ts[c]:
                    continue
                nc.tensor.matmul(banks[c][t][:, 2, :], lhsT=I100, rhs=t1_t[c],
                                 start=False, stop=True)

        def gates2(c, t):
            hh = gp.tile([100, BC], dt.bfloat16, tag=f"hh{c}", name=f"hh{c}",
                          bufs=2)
            nc.scalar.activation(hh, banks[c][t][0:100, 2, :], AF.Tanh)
            stw = stages[c][0:100, t, :]
            if t == 0:
                nc.vector.tensor_tensor(stw, zrs_t[c][:, 0, :], hh, OP.mult)
            else:
                # negw = (z - 1) * h_{t-1}: emitted HERE (not in gates1) so
                # on the DVE FIFO it sits directly ahead of this chunk's u,
                # running in the tanh shadow -- emitting all chunks' negw in
                # gates1 makes chunk 3's u queue behind 3 foreign negws
                ng = gp.tile([100, BC], dt.bfloat16, tag=f"ng{c}", name=f"ng{c}",
                             bufs=2)
                nc.vector.scalar_tensor_tensor(
                    ng, zrs_t[c][:, 0, :], 1.0, stages[c][0:100, t - 1, :],
                    OP.subtract, OP.mult)
                # h_t = u - negw,  u = z*hh  (two back-to-back DVE ops)
                u = gp.tile([100, BC], dt.bfloat16, tag=f"u{c}", name=f"u{c}",
                            bufs=2)
                nc.vector.tensor_tensor(u, zrs_t[c][:, 0, :], hh, OP.mult)
                nc.vector.tensor_tensor(stw, u, ng, OP.subtract)
            del banks[c][t]

        def finish_scan(c):
            tlast = Ts[c] - 1
            lo = gp.tile([100, BC], dt.float32, tag=f"lo{c}", name=f"lo{c}", bufs=1)
            nc.vector.tensor_copy(lo, stages[c][0:100, tlast, :])
            nc.sync.dma_start(out=lastout[c][:, :], in_=lo)

        # --- attention: chunk c processed in GL-step groups, reusing the
        #     retired chunks' PSUM pools; accumulator in SBUF fp32 on GpSimd.
        att = {}

        def att_start(c):
            # two accumulators: even groups accumulate on GpSimd, odd on DVE.
            # One Pool accumulator serializes at ~1.17us/group and saturates
            # the Pool (which shares its SBUF port with the DVE).
            acc = singles.tile([100, GL, BC], mybir.dt.float32, tag=f"accs{c}",
                               name=f"accs{c}")
            nc.vector.memset(acc, 0.0)
            accb = singles.tile([100, GL, BC], mybir.dt.float32, tag=f"accb{c}",
                                name=f"accb{c}")
            nc.vector.memset(accb, 0.0)
            # c1 = A1^T last, precomputed once, replicated into GL step slots
            c1p = bankp[c].tile([128, 4, BC], mybir.dt.float32, tag=f"b{c}",
                                name=f"c1p{c}")
            nc.tensor.matmul(c1p[:, 0, :], lhsT=A1,
                             rhs=stages[c][0:100, Ts[c] - 1, :],
                             start=True, stop=True)
            c1s = singles.tile([100, GL, BC], mybir.dt.bfloat16, tag=f"c1s{c}",
                               name=f"c1s{c}")
            for j in range(GL):
                nc.vector.tensor_copy(c1s[:, j, :], c1p[0:100, 0, :])
            # late chunks rotate over every already-idle pool so the drain
            # pipeline runs as many PSUM banks deep as are free
            partners = {0: [], 1: [0], 2: [1, 0], 3: [0, 1, 2]}[c]
            pools = [(bankp[c], f"b{c}")] + [(bankp[p], f"b{p}") for p in partners]
            att[c] = {"g": 0, "n": Ts[c] // GL, "pools": pools,
                      "acc": acc, "accb": accb, "c1s": c1s, "pend": []}

        def att_done(c):
            return c in att and att[c]["g"] >= att[c]["n"] and not att[c]["pend"]

        def att_stage1(c):
            """A2+c1 matmuls and the sigmoid for the next group."""
            stt = att[c]
            g = stt["g"]
            st4 = stages[c][0:100, GL * g:GL * (g + 1), :]
            pool_, tag_ = stt["pools"][g % len(stt["pools"])]
            sbal = pool_.tile([128, 4, BC], mybir.dt.float32,
                              tag=tag_, name=f"sbal{c}")
            # bank = A2^T st4 + c1 (c1 added on the PE via identity-accumulate)
            nc.tensor.matmul(sbal[:, :, :], lhsT=A2, rhs=st4, start=True, stop=False)
            nc.tensor.matmul(sbal[:, :, :], lhsT=I100, rhs=stt["c1s"],
                             start=False, stop=True)
            g2 = gp.tile([100, GL, BC], mybir.dt.bfloat16, tag=f"g{c}", name=f"g{c}",
                         bufs=2)
            nc.scalar.activation(g2, sbal[0:100, :, :], AF.Sigmoid)
            stt["pend"].append((sbal, g2, st4))
            stt["g"] = g + 1

        def att_stage2(c):
            """alpha matmul + alpha*state + accumulate for the oldest group."""
            stt = att[c]
            stt["r"] = stt.get("r", 0) + 1
            sbal, g2, st4 = stt["pend"].pop(0)
            # alpha (broadcast over partitions) overwrites the same bank
            nc.tensor.matmul(sbal[:, :, :], lhsT=Vr, rhs=g2, start=True, stop=True)
            tmp = gp.tile([100, GL, BC], mybir.dt.bfloat16, tag=f"tmp{c}", name=f"tmp{c}",
                          bufs=2)
            nc.vector.tensor_tensor(tmp, sbal[0:100, :, :], st4, OP.mult)
            if stt["r"] % 2:
                nc.gpsimd.tensor_tensor(stt["acc"], stt["acc"], tmp, OP.add)
            else:
                nc.vector.tensor_tensor(stt["accb"], stt["accb"], tmp, OP.add)
            if stt["g"] >= stt["n"] and not stt["pend"]:
                r2 = gp.tile([100, 2, BC], mybir.dt.float32, tag=f"r2{c}", name=f"r2{c}",
                             bufs=1)
                nc.vector.tensor_tensor(r2, stt["acc"][:, 0:2, :], stt["acc"][:, 2:4, :],
                                        OP.add)
                r2b = gp.tile([100, 2, BC], mybir.dt.float32, tag=f"r2b{c}",
                              name=f"r2b{c}", bufs=1)
                nc.vector.tensor_tensor(r2b, stt["accb"][:, 0:2, :],
                                        stt["accb"][:, 2:4, :], OP.add)
                nc.vector.tensor_tensor(r2, r2, r2b, OP.add)
                osum = gp.tile([100, BC], mybir.dt.float32, tag=f"os{c}", name=f"os{c}",
                               bufs=1)
                nc.vector.tensor_tensor(osum, r2[:, 0, :], r2[:, 1, :], OP.add)
                nc.sync.dma_start(out=outp[c][:, :], in_=osum)

        def att_try_starts(t):
            for c in ORDER:
                if c not in att and t >= Ts[c]:
                    att_start(c)

        def att_pump(budget, depth=2):
            """budget is in STAGES (1 stage = 1-2 PE matmuls), so scan-time
            pumping injects small slugs of PE work that fit the recurrence's
            idle gaps instead of whole groups that delay the next rec."""
            for c in ORDER:
                if c not in att:
                    continue
                stt = att[c]
                while budget > 0:
                    # stage1 first: emitting a retire (Vr) ahead of the next
                    # group's A2 head-of-line blocks the PE on the sigmoid
                    if stt["g"] < stt["n"] and len(stt["pend"]) < depth:
                        att_stage1(c)
                        budget -= 1
                    elif stt["pend"]:
                        att_stage2(c)
                        budget -= 1
                    else:
                        break

        # ---------------- emission ----------------
        for c in ORDER:
            issue_xdma(c, 0)
            issue_xdma(c, 1)
        xalloc(0)
        xmms(0, ORDER)

        for t in range(maxT):
            nact = sum(1 for c in range(NCHUNK) if t < Ts[c])
            # rec matmuls first: x(t+1) matmuls carry a WAR wait on the t-1
            # bank's readers, so putting them ahead of the recs couples the
            # longest chunk's chain to the other chunks' tanh reads
            recgroup(t)
            for c in ORDER:
                if t % 8 == 0 and t >= 8:
                    issue_xdma(c, t // 8 + 1)
            if t + 1 < maxT:
                act_next = xalloc(t + 1)
                xmms(t + 1, act_next)
            for c in ORDER:
                if t < Ts[c]:
                    gates1(c, t)
            iacc_group(t)
            for c in ORDER:
                if t < Ts[c]:
                    gates2(c, t)
                    if t == Ts[c] - 1:
                        finish_scan(c)
            att_try_starts(t)
            att_pump({4: 0, 3: 1, 2: 2, 1: 3}.get(nact, 3))

        while not all(att_done(c) for c in range(NCHUNK)):
            att_try_starts(10 ** 9)
            att_pump(6, depth=3)

    nc.compile()
    return nc


def _prep_weights(kernel_w, rec_kernel, bias_, A1_w, A2_w, v):
    b0, b1 = bias_[0], bias_[1]
    wall = np.zeros((128, 12, 128), np.float32)
    wall[:E, 0, :U] = -kernel_w[:, :U]
    wall[100, 0, :U] = -40.0
    wall[101, 0, :U] = -(b0[:U] + b1[:U])
    wall[:E, 1, :U] = kernel_w[:, U:2 * U]
    wall[101, 1, :U] = b0[U:2 * U] + b1[U:2 * U]
    wall[:E, 2, :U] = kernel_w[:, 2 * U:]
    wall[101, 2, :U] = b0[2 * U:]
    wall[:U, 3, :U] = -rec_kernel[:, :U]
    wall[:U, 4, :U] = rec_kernel[:, U:2 * U]
    wall[:U, 5, :U] = rec_kernel[:, 2 * U:]
    wall[100, 5, :U] = b1[2 * U:]
    wall[:U, 6, :U] = A1_w
    wall[:U, 7, :U] = A2_w
    wall[:U, 8, :U] = np.broadcast_to(v[0][:, None], (U, U))
    wall[:U, 9, :U] = np.eye(U, dtype=np.float32)
    wall[0, 10, :U] = b1[2 * U:]
    wall[:U, 11, 0] = b1[2 * U:]
    return {"wall": wall.astype(bf16),
            "wones": np.ones((1, T, BC), bf16)}


def kernel(session_hidden, mask, kernel, rec_kernel, bias, A1_w, A2_w, v):
    session_hidden = np.asarray(session_hidden, np.float32)
    mask = np.asarray(mask, np.float32)
    kernel_w = np.asarray(kernel, np.float32)
    rec_kernel = np.asarray(rec_kernel, np.float32)
    bias_ = np.asarray(bias, np.float32)
    A1_w = np.asarray(A1_w, np.float32)
    A2_w = np.asarray(A2_w, np.float32)
    v = np.asarray(v, np.float32)

    lengths = mask.sum(1).astype(np.int64)  # in [1, T]
    order = np.argsort(lengths, kind="stable")
    # deal round-robin: sorted rank i -> core i%8, slot i//8
    slot = np.arange(B) // NCORES
    core = np.arange(B) % NCORES
    perm = np.empty(B, np.int64)
    perm[core * PERCORE + slot] = order  # arranged[core*512+slot] = orig row
    lens_a = lengths[perm]
    lens_sorted = lengths[order]
    Ts = tuple(_ceil8(lens_sorted[NCORES * BC * (c + 1) - 1])
               for c in range(NCHUNK))

    key = Ts
    if key not in _CACHE:
        _CACHE[key] = _build(Ts)
    nc = _CACHE[key]
    _CACHE["nc"] = nc

    w = _prep_weights(kernel_w, rec_kernel, bias_, A1_w, A2_w, v)

    x_a = session_hidden[perm].reshape(NCORES, NCHUNK, BC, T, E)
    m_a = mask[perm].reshape(NCORES, NCHUNK, BC, T)
    in_maps = []
    for k in range(NCORES):
        im = dict(w)
        for c in range(NCHUNK):
            Tc = Ts[c]
            xc = np.zeros((128, Tc, BC), np.float32)
            xc[:E] = x_a[k, c, :, :Tc, :].transpose(2, 1, 0)
            xc[100] = 1.0 - m_a[k, c, :, :Tc].transpose(1, 0)
            xc[101] = 1.0
            im[f"xc{c}"] = xc.astype(bf16)
        in_maps.append(im)

    _CACHE["in_maps"] = in_maps
    res = bass_utils.run_bass_kernel_spmd(nc, in_maps, core_ids=list(range(NCORES)))

    out_dev = np.zeros((B, U), np.float32)
    last = np.zeros((B, U), np.float32)
    for k in range(NCORES):
        r = res.results[k]
        for c in range(NCHUNK):
            sl_ = slice(k * PERCORE + c * BC, k * PERCORE + (c + 1) * BC)
            out_dev[sl_] = np.asarray(r[f"outp{c}"]).T.astype(np.float32)
            last[sl_] = np.asarray(r[f"lastc{c}"]).T.astype(np.float32)

    # host correction: device ran steps [0, T_c) with the A1*last term for all t.
    # truth: masked t in [len, T) contribute sigmoid(A2^T last)@v * last.
    Tc_a = np.tile(np.repeat(np.asarray(Ts, np.float32), BC), NCORES)
    sl_ = last @ A2_w
    c_ = last @ A1_w
    sig = lambda a: 1.0 / (1.0 + np.exp(-a))
    a1 = sig(sl_ + c_) @ v[0]
    a0 = sig(sl_) @ v[0]
    lf = lens_a.astype(np.float32)
    out_a = out_dev - (Tc_a - lf)[:, None] * a1[:, None] * last \
        + (T - lf)[:, None] * a0[:, None] * last

    out = np.empty((B, U), np.float32)
    out[perm] = out_a
    _CACHE["debug"] = dict(out_dev=out_dev, last=last, perm=perm, Ts=Ts,
                           lens_a=lens_a, out_a=out_a)
    return out.astype(np.float32)
